# revision 15
# baseline (speedup 1.0000x reference)
"""nn_DGAttention Trainium2 Bass kernel (8 NeuronCores, data-parallel over batch).

Contract: kernel(**inputs) takes the FULL unsharded inputs (feat0/1/2 [512,256],
Wq/Wk/Wv [3,256,256], bq/bk/bv [3,256], gamma [3]) and returns the FULL output
(ret_feats [3,512,256], alphas [3]), matching reference().

Strategy (per core, 64 batch rows):
  Q_i = concat_j relu(F_j @ Wq_i.T + bq_i)   [64, 768]
  K_i = relu(F_i @ Wk_i.T + bk_i)            [64, 256]
  V_i = relu(F_i @ Wv_i.T + bv_i)            [64, 256]
  per (i, b): E^T[q, jp] = K_i[b,q] * Q_i[b,jp] as 4 K=1 outer-product matmuls
  into PSUM; one Exp activation [128,1536] PSUM->SBUF (fp32r); then an M=2
  matmul with lhsT = [ones | V_i^T[:,b]] contracts over q to give Z and W rows;
  out_i[b,p] = sum_j W/Z; feats_i = out_i * gamma_i/3 + F_i.
  Softmax over q is computed without max-subtraction (energies are bounded,
  exp stays in fp32 range; the ratio is mathematically identical).
  means[i] = sum_b rowsum(Q_i) * rowsum(K_i) -> host reduce -> softmax -> alphas.

Batch row b maps to (g, r) = (b % 4, b // 4); Q/K rows live on SBUF partition
32g (32-aligned base partitions are required for K=1 matmul operands), bounced
through DRAM to perform the partition scatter + fp32r cast.
"""

import numpy as np
import concourse.bass as bass
import concourse.tile as tile
from concourse import bacc, mybir
from concourse.bass_utils import run_bass_kernel_spmd

F32 = mybir.dt.float32
F32R = mybir.dt.float32r
AF = mybir.ActivationFunctionType

B = 512
D = 256
NJ = 3
JP = NJ * D  # 768
N_CORES = 8
NB = B // N_CORES  # 64 batch rows per core
NR = NB // 4


def _emit(nc, tc, pools, aps, gamma):
    io, pro, mainq, expp, stg, epi, psA, psB, dram = pools
    (ftp_d, wq1_d, wk1_d, wv1_d, fnat_d, vinit_d, feats_d, meansp_d) = aps

    ftp = io.tile([128, 3 * NJ * NB], F32R, tag="ftp", name="ftp")
    nc.sync.dma_start(ftp[:], ftp_d[:])
    ftp3 = ftp[:].rearrange("p (t b) -> p t b", b=NB)
    w1 = {}
    qs = [nc.sync, nc.scalar]
    for qi, (name, dten) in enumerate((("k", wk1_d), ("q", wq1_d), ("v", wv1_d))):
        t = io.tile([128, 3 * NJ * D], F32R, tag=f"w{name}", name=f"w1{name}")
        t3 = t[:].rearrange("p (t e) -> p t e", e=D)
        d3 = dten.rearrange("p t e -> p t e")
        for i in range(NJ):
            qs[(qi + i) % 2].dma_start(t3[:, 3 * i:3 * i + 3, :],
                                       d3[:, 3 * i:3 * i + 3, :])
        w1[name] = t3

    # DRAM bounce in [r, g, f] order: SBUF store iterates r (partitions)
    # outermost, load iterates g (partitions) outermost; DRAM AP reorders.
    qpack_dram = dram.tile([NJ, NR, 4, JP], F32R, tag="qpd", name="qpack_dram")
    kpack_dram = dram.tile([NJ, NR, 4, D], F32R, tag="kpd", name="kpack_dram")

    # vones[i][h]: [128, 2*NB]; col 2b = 1.0, col 2b+1 = V_i^T[128h+q, b]
    vones = [[io.tile([128, 2 * NB], F32R, tag=f"vo{i}{h}", name=f"vones{i}{h}")
              for h in range(2)] for i in range(NJ)]

    def prologue(i):
        # Per-half (gg covers partition groups 2gg, 2gg+1) K then Q paths so
        # the first half reaches DRAM (and the main loop) as early as possible.
        kstage = pro.tile([NR, 4 * D], F32, tag="kstage", name="kstage")
        qstage = pro.tile([NR, 4 * JP], F32, tag="qstage", name="qstage")
        for gg in range(2):
            psk = psA.tile([NR, 2 * D], F32, tag="big", name="psk")
            for g2 in range(2):
                g = 2 * gg + g2
                for t in range(2):
                    nc.tensor.matmul(
                        psk[:, g2 * D:(g2 + 1) * D],
                        ftp3[:, 3 * i + t, g::4],
                        w1["k"][:, 3 * i + t, :],
                        start=(t == 0), stop=False)
                nc.tensor.matmul(
                    psk[:, g2 * D:(g2 + 1) * D],
                    ftp3[0:1, 3 * i + 2, g::4],
                    w1["k"][0:1, 3 * i + 2, :],
                    start=False, stop=True)
            nc.vector.tensor_scalar_max(
                kstage[:, gg * 2 * D:(gg + 1) * 2 * D], psk[:, :], 0.0)
            nc.gpsimd.dma_start(
                kpack_dram[i, :, 2 * gg:2 * gg + 2, :],
                kstage[:, gg * 2 * D:(gg + 1) * 2 * D]
                .rearrange("r (g f) -> r g f", g=2))

            psq = psA.tile([NR, 2 * JP], F32, tag="big", name="psq")
            for g2 in range(2):
                g = 2 * gg + g2
                for j in range(NJ):
                    col = g2 * JP + j * D
                    for t in range(2):
                        nc.tensor.matmul(
                            psq[:, col:col + D],
                            ftp3[:, 3 * j + t, g::4],
                            w1["q"][:, 3 * i + t, :],
                            start=(t == 0), stop=False)
                    nc.tensor.matmul(
                        psq[:, col:col + D],
                        ftp3[0:1, 3 * j + 2, g::4],
                        w1["q"][0:1, 3 * i + 2, :],
                        start=False, stop=True)
            nc.vector.tensor_scalar_max(
                qstage[:, gg * 2 * JP:(gg + 1) * 2 * JP], psq[:, :], 0.0)
            nc.gpsimd.dma_start(
                qpack_dram[i, :, 2 * gg:2 * gg + 2, :],
                qstage[:, gg * 2 * JP:(gg + 1) * 2 * JP]
                .rearrange("r (g f) -> r g f", g=2))

        # ---- V_i^T into vones odd columns (fp32r-rounded by ACT) ----
        for h in range(2):
            nc.sync.dma_start(vones[i][h][:], vinit_d[:])
            psv = psA.tile([128, NB], F32, tag="big", name="psv")
            for t in range(2):
                nc.tensor.matmul(
                    psv[:, :],
                    w1["v"][:, 3 * i + t, 128 * h:128 * h + 128],
                    ftp3[:, 3 * i + t, :],
                    start=(t == 0), stop=False)
            nc.tensor.matmul(
                psv[:, :],
                w1["v"][0:1, 3 * i + 2, 128 * h:128 * h + 128],
                ftp3[0:1, 3 * i + 2, :],
                start=False, stop=True)
            nc.vector.tensor_scalar_max(vones[i][h][:, 1::2], psv[:, :], 0.0)

        # ---- means partials: rowsum(Q)*rowsum(K) per (r, g) -> meansp[i, b]
        qrs = pro.tile([NR, 4], F32, tag="qrs", name="qrs")
        nc.vector.reduce_sum(
            qrs[:], qstage[:].rearrange("r (g f) -> r g f", g=4),
            axis=mybir.AxisListType.X)
        krs = pro.tile([NR, 4], F32, tag="krs", name="krs")
        nc.vector.reduce_sum(
            krs[:], kstage[:].rearrange("r (g f) -> r g f", g=4),
            axis=mybir.AxisListType.X)
        mp = pro.tile([NR, 4], F32, tag="mp", name="mp")
        nc.vector.tensor_mul(mp[:], qrs[:], krs[:])
        nc.sync.dma_start(meansp_d[i, 0:4 * NR], mp[:])

    # ================= main: attention =================
    # etile columns: [0:512]=h0 jp0:512, [512:768]=h0 jp512:768,
    #                [768:1024]=h1 jp0:256, [1024:1536]=h1 jp256:768
    OUTER_CHUNKS = [(0, 0, 512, 0), (0, 512, 256, 512),
                    (1, 0, 256, 768), (1, 256, 512, 1024)]
    QRT = 32  # rows per epilogue chunk (partition starts must be 32-aligned)

    def main_phase(i, inject=None):
        # qpack/kpack loads split per half: the gg half of the NEXT phase can
        # load while this phase is still consuming the other half's partitions.
        qpack = mainq.tile([128, NR * JP], F32R, tag="qpack", name="qpack",
                           bufs=1)
        kpack = mainq.tile([128, NR * D], F32R, tag="kpack", name="kpack",
                           bufs=1)
        for gg in range(2):
            [nc.sync, nc.scalar][gg].dma_start(
                qpack[64 * gg:64 * gg + 33:32, :]
                .rearrange("g (r f) -> g r f", r=NR),
                qpack_dram[i, :, 2 * gg:2 * gg + 2, :]
                .rearrange("r g f -> g r f"))
            [nc.scalar, nc.sync][gg].dma_start(
                kpack[64 * gg:64 * gg + 33:32, :]
                .rearrange("g (r f) -> g r f", r=NR),
                kpack_dram[i, :, 2 * gg:2 * gg + 2, :]
                .rearrange("r g f -> g r f"))

        zwbuf = epi.tile([NB, 2 * JP], F32, tag="zwbuf", name="zwbuf", bufs=2)
        fn = epi.tile([NB, D], F32, tag="fn", name="fn", bufs=2)
        nc.sync.dma_start(fn[:], fnat_d[i, :, :])

        def pair(n, g, r):
            et = psA.tile([128, 1536], F32, tag="big", name="et")
            for (h, jp0, w, ec) in OUTER_CHUNKS:
                nc.tensor.matmul(
                    et[:, ec:ec + w],
                    kpack[32 * g:32 * g + 1,
                          D * r + 128 * h:D * r + 128 * h + 128],
                    qpack[32 * g:32 * g + 1,
                          JP * r + jp0:JP * r + jp0 + w],
                    start=True, stop=True,
                    tile_position=(32 * g, 0))
            ex = expp.tile([128, 1536], F32R, tag="expE", name="ex", bufs=3)
            nc.scalar.activation(ex[:], et[:], AF.Exp)
            lhs = [vones[i][h][:, 2 * n:2 * n + 2] for h in range(2)]
            zwA = psB.tile([2, 512], F32, tag="zwA", name="zwA")
            nc.tensor.matmul(zwA[:], lhs[0], ex[:, 0:512],
                             start=True, stop=False, skip_group_check=True)
            nc.tensor.matmul(zwA[:], lhs[1], ex[:, 768:1280],
                             start=False, stop=True, skip_group_check=True)
            zwB = psB.tile([2, 256], F32, tag="zwB", name="zwB")
            nc.tensor.matmul(zwB[:], lhs[0], ex[:, 512:768],
                             start=True, stop=False, skip_group_check=True)
            nc.tensor.matmul(zwB[:], lhs[1], ex[:, 1280:1536],
                             start=False, stop=True, skip_group_check=True)
            return zwA, zwB

        def epilogue_chunk(q):
            # full-height temps sliced at the same rows: SB two-input ops
            # require equal start partitions on both operands
            r0 = QRT * q
            sl = slice(r0, r0 + QRT)
            rz = epi.tile([NB, JP], F32, tag="rz", name="rz", bufs=2)
            nc.vector.reciprocal(rz[sl, :], zwbuf[sl, 0:JP])
            rr = epi.tile([NB, JP], F32, tag="rr", name="rr", bufs=2)
            nc.vector.tensor_mul(rr[sl, :], zwbuf[sl, JP:2 * JP], rz[sl, :])
            s1 = epi.tile([NB, D], F32, tag="s1", name="s1", bufs=2)
            nc.vector.tensor_add(s1[sl, :], rr[sl, 0:D], rr[sl, D:2 * D])
            s2 = epi.tile([NB, D], F32, tag="s2", name="s2", bufs=2)
            nc.vector.tensor_add(s2[sl, :], s1[sl, :], rr[sl, 2 * D:3 * D])
            ot = epi.tile([NB, D], F32, tag="ot", name="ot", bufs=2)
            nc.vector.tensor_scalar_mul(ot[sl, :], s2[sl, :],
                                        float(gamma[i]) / 3.0)
            fo = epi.tile([NB, D], F32, tag="fo", name="fo", bufs=2)
            nc.vector.tensor_add(fo[sl, :], ot[sl, :], fn[sl, :])
            nc.sync.dma_start(feats_d[i, sl, :], fo[sl, :])

        # sections by gg: first consume partition groups {0,1}, then {2,3};
        # stage groups of 4 pairs = (g in section) x (r, r+1).
        # Outer products are emitted one pair ahead of their exp so the PE
        # stream runs [outer(n+1), ZW(n)] inside exp(n)'s shadow.
        seq = []
        for gg in range(2):
            for u in range(NR // 2):
                for k in range(4):
                    g2, dr = k % 2, k // 2
                    g, r = 2 * gg + g2, 2 * u + dr
                    seq.append((g, r))

        def emit_outer(idx):
            g, r = seq[idx]
            et = psA.tile([128, 1536], F32, tag="big", name="et")
            for (h, jp0, w, ec) in OUTER_CHUNKS:
                nc.tensor.matmul(
                    et[:, ec:ec + w],
                    kpack[32 * g:32 * g + 1,
                          D * r + 128 * h:D * r + 128 * h + 128],
                    qpack[32 * g:32 * g + 1,
                          JP * r + jp0:JP * r + jp0 + w],
                    start=True, stop=True,
                    tile_position=(32 * g, 0))
            return et

        pend = {0: emit_outer(0)}
        exs = {}
        stages = {}

        def emit_zw(jdx):
            # ZW + copies for pair jdx (one iteration behind its exp, so the
            # PE stream meets each exp's outer products first)
            g, r = seq[jdx]
            n = 4 * r + g
            k = jdx % 4
            if k == 0:
                stages[jdx // 4] = stg.tile([2, 4 * JP], F32, tag="stage",
                                            name="stage")
            stage = stages[jdx // 4]
            ex = exs.pop(jdx)
            lhs = [vones[i][h][:, 2 * n:2 * n + 2] for h in range(2)]
            zwA = psB.tile([2, 512], F32, tag="zwA", name="zwA")
            nc.tensor.matmul(zwA[:], lhs[0], ex[:, 0:512],
                             start=True, stop=False, skip_group_check=True)
            nc.tensor.matmul(zwA[:], lhs[1], ex[:, 768:1280],
                             start=False, stop=True, skip_group_check=True)
            zwB = psB.tile([2, 256], F32, tag="zwB", name="zwB")
            nc.tensor.matmul(zwB[:], lhs[0], ex[:, 512:768],
                             start=True, stop=False, skip_group_check=True)
            nc.tensor.matmul(zwB[:], lhs[1], ex[:, 1280:1536],
                             start=False, stop=True, skip_group_check=True)
            nc.vector.tensor_copy(stage[:, JP * k:JP * k + 512], zwA[:])
            nc.vector.tensor_copy(stage[:, JP * k + 512:JP * k + 768], zwB[:])
            if k == 3:
                gg, u = jdx // 32, (jdx % 32) // 4
                base = 8 * u + 2 * gg
                for c in range(2):
                    nc.sync.dma_start(
                        zwbuf[base + 4 * c:base + 4 * c + 2, 0:JP],
                        stage[0:1, 2 * c * JP:(2 * c + 2) * JP]
                        .rearrange("p (k f) -> p k f", k=2))
                    nc.sync.dma_start(
                        zwbuf[base + 4 * c:base + 4 * c + 2, JP:2 * JP],
                        stage[1:2, 2 * c * JP:(2 * c + 2) * JP]
                        .rearrange("p (k f) -> p k f", k=2))
                del stages[jdx // 4]
                if gg == 1 and u % 4 == 3:
                    epilogue_chunk(u // 4)
                if inject is not None and jdx == 39:
                    inject()

        for idx in range(len(seq)):
            et = pend.pop(idx)
            ex = expp.tile([128, 1536], F32R, tag="expE", name="ex", bufs=3)
            nc.scalar.activation(ex[:], et[:], AF.Exp)
            exs[idx] = ex
            if idx + 1 < len(seq):
                pend[idx + 1] = emit_outer(idx + 1)
            if idx >= 1:
                emit_zw(idx - 1)
        emit_zw(len(seq) - 1)

    prologue(0)
    for i in range(NJ):
        nxt = (lambda j: (lambda: prologue(j)))(i + 1) if i + 1 < NJ else None
        main_phase(i, inject=nxt)


def build_nc(gamma, repeat=1):
    nc = bacc.Bacc("TRN2", target_bir_lowering=False, debug=False,
                   num_devices=N_CORES)
    ftp_d = nc.dram_tensor("ftp", [128, 3 * NJ, NB], F32R, kind="ExternalInput").ap()
    wq1_d = nc.dram_tensor("wq1", [128, 3 * NJ, D], F32R, kind="ExternalInput").ap()
    wk1_d = nc.dram_tensor("wk1", [128, 3 * NJ, D], F32R, kind="ExternalInput").ap()
    wv1_d = nc.dram_tensor("wv1", [128, 3 * NJ, D], F32R, kind="ExternalInput").ap()
    fnat_d = nc.dram_tensor("fnat", [NJ, NB, D], F32, kind="ExternalInput").ap()
    vinit_d = nc.dram_tensor("vinit", [128, 2 * NB], F32R, kind="ExternalInput").ap()
    feats_d = nc.dram_tensor("feats", [NJ, NB, D], F32, kind="ExternalOutput").ap()
    meansp_d = nc.dram_tensor("meansp", [NJ, 128], F32, kind="ExternalOutput").ap()
    aps = (ftp_d, wq1_d, wk1_d, wv1_d, fnat_d, vinit_d, feats_d, meansp_d)

    with tile.TileContext(nc) as tc:
        with tc.tile_pool(name="io", bufs=1) as io, \
             tc.tile_pool(name="pro", bufs=1) as pro, \
             tc.tile_pool(name="mainq", bufs=1) as mainq, \
             tc.tile_pool(name="expp", bufs=2) as expp, \
             tc.tile_pool(name="stg", bufs=2) as stg, \
             tc.tile_pool(name="epi", bufs=1) as epi, \
             tc.tile_pool(name="psA", bufs=2, space="PSUM") as psA, \
             tc.tile_pool(name="psB", bufs=1, space="PSUM") as psB, \
             tc.tile_pool(name="dram", bufs=1, space="DRAM") as dram:
            pools = (io, pro, mainq, expp, stg, epi, psA, psB, dram)
            for _ in range(repeat):
                _emit(nc, tc, pools, aps, gamma)
    nc.compile()
    return nc


def pack_inputs(feats_list, Wq, bq, Wk, bk, Wv, bv, b0):
    ftp = np.zeros((128, 3 * NJ, NB), np.float32)
    for j in range(NJ):
        FjT = feats_list[j][b0:b0 + NB].T
        ftp[:, 3 * j + 0, :] = FjT[0:128]
        ftp[:, 3 * j + 1, :] = FjT[128:256]
        ftp[0, 3 * j + 2, :] = 1.0

    def w1(W, b):
        out = np.zeros((128, 3 * NJ, D), np.float32)
        for i in range(NJ):
            WT = W[i].T
            out[:, 3 * i + 0, :] = WT[0:128]
            out[:, 3 * i + 1, :] = WT[128:256]
            out[0, 3 * i + 2, :] = b[i]
        return out

    fnat = np.stack([f[b0:b0 + NB] for f in feats_list]).astype(np.float32)
    return {
        "ftp": ftp.reshape(128, -1),
        "wq1": w1(Wq, bq).reshape(128, -1),
        "wk1": w1(Wk, bk).reshape(128, -1),
        "wv1": w1(Wv, bv).reshape(128, -1),
        "fnat": fnat,
        "vinit": np.ones((128, 2 * NB), np.float32),
    }


def postprocess(results):
    feats = np.concatenate([np.asarray(r["feats"]) for r in results], axis=1)
    tot = np.zeros(NJ, np.float64)
    for r in results:
        tot += np.asarray(r["meansp"])[:, 0:NB].astype(np.float64).sum(axis=1)
    means = tot / (NJ * B * D * D)
    e = np.exp(means - means.max())
    alphas = (e / e.sum()).astype(np.float32)
    return np.ascontiguousarray(feats.astype(np.float32)), alphas


_nc_cache = {}


def get_nc(gamma, repeat=1):
    key = (bytes(np.asarray(gamma, np.float32).tobytes()), repeat)
    if key not in _nc_cache:
        _nc_cache[key] = build_nc(gamma, repeat=repeat)
    return _nc_cache[key]


def kernel(feat0, feat1, feat2, Wq, bq, Wk, bk, Wv, bv, gamma):
    feats_list = [np.asarray(feat0, np.float32), np.asarray(feat1, np.float32),
                  np.asarray(feat2, np.float32)]
    Wq, bq = np.asarray(Wq, np.float32), np.asarray(bq, np.float32)
    Wk, bk = np.asarray(Wk, np.float32), np.asarray(bk, np.float32)
    Wv, bv = np.asarray(Wv, np.float32), np.asarray(bv, np.float32)
    gamma = np.asarray(gamma, np.float32)

    nc = get_nc(gamma)
    in_maps = [pack_inputs(feats_list, Wq, bq, Wk, bk, Wv, bv, NB * c)
               for c in range(N_CORES)]
    res = run_bass_kernel_spmd(nc, in_maps, core_ids=list(range(N_CORES)))
    return postprocess(res.results)


# revision 16
# speedup vs baseline: 1.0107x; 1.0107x over previous
"""nn_DGAttention Trainium2 Bass kernel (8 NeuronCores, data-parallel over batch).

Contract: kernel(**inputs) takes the FULL unsharded inputs (feat0/1/2 [512,256],
Wq/Wk/Wv [3,256,256], bq/bk/bv [3,256], gamma [3]) and returns the FULL output
(ret_feats [3,512,256], alphas [3]), matching reference().

Strategy (per core, 64 batch rows):
  Q_i = concat_j relu(F_j @ Wq_i.T + bq_i)   [64, 768]
  K_i = relu(F_i @ Wk_i.T + bk_i)            [64, 256]
  V_i = relu(F_i @ Wv_i.T + bv_i)            [64, 256]
  per (i, b): E^T[q, jp] = K_i[b,q] * Q_i[b,jp] as 4 K=1 outer-product matmuls
  into PSUM; one Exp activation [128,1536] PSUM->SBUF (fp32r); then an M=2
  matmul with lhsT = [ones | V_i^T[:,b]] contracts over q to give Z and W rows;
  out_i[b,p] = sum_j W/Z; feats_i = out_i * gamma_i/3 + F_i.
  Softmax over q is computed without max-subtraction (energies are bounded,
  exp stays in fp32 range; the ratio is mathematically identical).
  means[i] = sum_b rowsum(Q_i) * rowsum(K_i) -> host reduce -> softmax -> alphas.

Batch row b maps to (g, r) = (b % 4, b // 4); Q/K rows live on SBUF partition
32g (32-aligned base partitions are required for K=1 matmul operands), bounced
through DRAM to perform the partition scatter + fp32r cast.
"""

import numpy as np
import concourse.bass as bass
import concourse.tile as tile
from concourse import bacc, mybir
from concourse.bass_utils import run_bass_kernel_spmd

F32 = mybir.dt.float32
F32R = mybir.dt.float32r
AF = mybir.ActivationFunctionType

B = 512
D = 256
NJ = 3
JP = NJ * D  # 768
N_CORES = 8
NB = B // N_CORES  # 64 batch rows per core
NR = NB // 4


def _emit(nc, tc, pools, aps, gamma):
    io, pro, mainq, expp, stg, epi, psA, psB, dram = pools
    (ftp_d, wq1_d, wk1_d, wv1_d, fnat_d, vinit_d, feats_d, meansp_d) = aps

    ftp = io.tile([128, 3 * NJ * NB], F32R, tag="ftp", name="ftp")
    nc.sync.dma_start(ftp[:], ftp_d[:])
    ftp3 = ftp[:].rearrange("p (t b) -> p t b", b=NB)
    w1 = {}
    qs = [nc.sync, nc.scalar]
    for qi, (name, dten) in enumerate((("k", wk1_d), ("q", wq1_d), ("v", wv1_d))):
        t = io.tile([128, 3 * NJ * D], F32R, tag=f"w{name}", name=f"w1{name}")
        t3 = t[:].rearrange("p (t e) -> p t e", e=D)
        d3 = dten.rearrange("p t e -> p t e")
        for i in range(NJ):
            qs[(qi + i) % 2].dma_start(t3[:, 3 * i:3 * i + 3, :],
                                       d3[:, 3 * i:3 * i + 3, :])
        w1[name] = t3

    # DRAM bounce in [r, g, f] order: SBUF store iterates r (partitions)
    # outermost, load iterates g (partitions) outermost; DRAM AP reorders.
    qpack_dram = dram.tile([NJ, NR, 4, JP], F32R, tag="qpd", name="qpack_dram")
    kpack_dram = dram.tile([NJ, NR, 4, D], F32R, tag="kpd", name="kpack_dram")

    # vones[i][h]: [128, 2*NB]; col 2b = 1.0, col 2b+1 = V_i^T[128h+q, b]
    vones = [[io.tile([128, 2 * NB], F32R, tag=f"vo{i}{h}", name=f"vones{i}{h}")
              for h in range(2)] for i in range(NJ)]

    def prologue(i):
        # Per-half (gg covers partition groups 2gg, 2gg+1) K then Q paths so
        # the first half reaches DRAM (and the main loop) as early as possible.
        kstage = pro.tile([NR, 4 * D], F32, tag="kstage", name="kstage")
        qstage = pro.tile([NR, 4 * JP], F32, tag="qstage", name="qstage")
        for gg in range(2):
            psk = psA.tile([NR, 2 * D], F32, tag="big", name="psk")
            for g2 in range(2):
                g = 2 * gg + g2
                for t in range(2):
                    nc.tensor.matmul(
                        psk[:, g2 * D:(g2 + 1) * D],
                        ftp3[:, 3 * i + t, g::4],
                        w1["k"][:, 3 * i + t, :],
                        start=(t == 0), stop=False)
                nc.tensor.matmul(
                    psk[:, g2 * D:(g2 + 1) * D],
                    ftp3[0:1, 3 * i + 2, g::4],
                    w1["k"][0:1, 3 * i + 2, :],
                    start=False, stop=True)
            nc.scalar.activation(
                kstage[:, gg * 2 * D:(gg + 1) * 2 * D], psk[:, :], AF.Relu)
            nc.gpsimd.dma_start(
                kpack_dram[i, :, 2 * gg:2 * gg + 2, :],
                kstage[:, gg * 2 * D:(gg + 1) * 2 * D]
                .rearrange("r (g f) -> r g f", g=2))

            psq = psA.tile([NR, 2 * JP], F32, tag="big", name="psq")
            for g2 in range(2):
                g = 2 * gg + g2
                for j in range(NJ):
                    col = g2 * JP + j * D
                    for t in range(2):
                        nc.tensor.matmul(
                            psq[:, col:col + D],
                            ftp3[:, 3 * j + t, g::4],
                            w1["q"][:, 3 * i + t, :],
                            start=(t == 0), stop=False)
                    nc.tensor.matmul(
                        psq[:, col:col + D],
                        ftp3[0:1, 3 * j + 2, g::4],
                        w1["q"][0:1, 3 * i + 2, :],
                        start=False, stop=True)
            nc.scalar.activation(
                qstage[:, gg * 2 * JP:(gg + 1) * 2 * JP], psq[:, :], AF.Relu)
            nc.gpsimd.dma_start(
                qpack_dram[i, :, 2 * gg:2 * gg + 2, :],
                qstage[:, gg * 2 * JP:(gg + 1) * 2 * JP]
                .rearrange("r (g f) -> r g f", g=2))

        # ---- V_i^T into vones odd columns (fp32r-rounded by ACT) ----
        for h in range(2):
            nc.sync.dma_start(vones[i][h][:], vinit_d[:])
            psv = psA.tile([128, NB], F32, tag="big", name="psv")
            for t in range(2):
                nc.tensor.matmul(
                    psv[:, :],
                    w1["v"][:, 3 * i + t, 128 * h:128 * h + 128],
                    ftp3[:, 3 * i + t, :],
                    start=(t == 0), stop=False)
            nc.tensor.matmul(
                psv[:, :],
                w1["v"][0:1, 3 * i + 2, 128 * h:128 * h + 128],
                ftp3[0:1, 3 * i + 2, :],
                start=False, stop=True)
            nc.scalar.activation(vones[i][h][:, 1::2], psv[:, :], AF.Relu)

        # ---- means partials: rowsum(Q)*rowsum(K) per (r, g) -> meansp[i, b]
        qrs = pro.tile([NR, 4], F32, tag="qrs", name="qrs")
        nc.vector.reduce_sum(
            qrs[:], qstage[:].rearrange("r (g f) -> r g f", g=4),
            axis=mybir.AxisListType.X)
        krs = pro.tile([NR, 4], F32, tag="krs", name="krs")
        nc.vector.reduce_sum(
            krs[:], kstage[:].rearrange("r (g f) -> r g f", g=4),
            axis=mybir.AxisListType.X)
        mp = pro.tile([NR, 4], F32, tag="mp", name="mp")
        nc.vector.tensor_mul(mp[:], qrs[:], krs[:])
        nc.sync.dma_start(meansp_d[i, 0:4 * NR], mp[:])

    # ================= main: attention =================
    # etile columns: [0:512]=h0 jp0:512, [512:768]=h0 jp512:768,
    #                [768:1024]=h1 jp0:256, [1024:1536]=h1 jp256:768
    OUTER_CHUNKS = [(0, 0, 512, 0), (0, 512, 256, 512),
                    (1, 0, 256, 768), (1, 256, 512, 1024)]
    QRT = 32  # rows per epilogue chunk (partition starts must be 32-aligned)

    def main_phase(i, inject=None):
        # qpack/kpack loads split per half: the gg half of the NEXT phase can
        # load while this phase is still consuming the other half's partitions.
        qpack = mainq.tile([128, NR * JP], F32R, tag="qpack", name="qpack",
                           bufs=1)
        kpack = mainq.tile([128, NR * D], F32R, tag="kpack", name="kpack",
                           bufs=1)
        for gg in range(2):
            [nc.sync, nc.scalar][gg].dma_start(
                qpack[64 * gg:64 * gg + 33:32, :]
                .rearrange("g (r f) -> g r f", r=NR),
                qpack_dram[i, :, 2 * gg:2 * gg + 2, :]
                .rearrange("r g f -> g r f"))
            [nc.scalar, nc.sync][gg].dma_start(
                kpack[64 * gg:64 * gg + 33:32, :]
                .rearrange("g (r f) -> g r f", r=NR),
                kpack_dram[i, :, 2 * gg:2 * gg + 2, :]
                .rearrange("r g f -> g r f"))

        zwbuf = epi.tile([NB, 2 * JP], F32, tag="zwbuf", name="zwbuf", bufs=2)
        fn = epi.tile([NB, D], F32, tag="fn", name="fn", bufs=2)
        nc.sync.dma_start(fn[:], fnat_d[i, :, :])

        def pair(n, g, r):
            et = psA.tile([128, 1536], F32, tag="big", name="et")
            for (h, jp0, w, ec) in OUTER_CHUNKS:
                nc.tensor.matmul(
                    et[:, ec:ec + w],
                    kpack[32 * g:32 * g + 1,
                          D * r + 128 * h:D * r + 128 * h + 128],
                    qpack[32 * g:32 * g + 1,
                          JP * r + jp0:JP * r + jp0 + w],
                    start=True, stop=True,
                    tile_position=(32 * g, 0))
            ex = expp.tile([128, 1536], F32R, tag="expE", name="ex", bufs=3)
            nc.scalar.activation(ex[:], et[:], AF.Exp)
            lhs = [vones[i][h][:, 2 * n:2 * n + 2] for h in range(2)]
            zwA = psB.tile([2, 512], F32, tag="zwA", name="zwA")
            nc.tensor.matmul(zwA[:], lhs[0], ex[:, 0:512],
                             start=True, stop=False, skip_group_check=True)
            nc.tensor.matmul(zwA[:], lhs[1], ex[:, 768:1280],
                             start=False, stop=True, skip_group_check=True)
            zwB = psB.tile([2, 256], F32, tag="zwB", name="zwB")
            nc.tensor.matmul(zwB[:], lhs[0], ex[:, 512:768],
                             start=True, stop=False, skip_group_check=True)
            nc.tensor.matmul(zwB[:], lhs[1], ex[:, 1280:1536],
                             start=False, stop=True, skip_group_check=True)
            return zwA, zwB

        def epilogue_chunk(q):
            # full-height temps sliced at the same rows: SB two-input ops
            # require equal start partitions on both operands
            r0 = QRT * q
            sl = slice(r0, r0 + QRT)
            rz = epi.tile([NB, JP], F32, tag="rz", name="rz", bufs=2)
            nc.vector.reciprocal(rz[sl, :], zwbuf[sl, 0:JP])
            rr = epi.tile([NB, JP], F32, tag="rr", name="rr", bufs=2)
            nc.vector.tensor_mul(rr[sl, :], zwbuf[sl, JP:2 * JP], rz[sl, :])
            s1 = epi.tile([NB, D], F32, tag="s1", name="s1", bufs=2)
            nc.vector.tensor_add(s1[sl, :], rr[sl, 0:D], rr[sl, D:2 * D])
            s2 = epi.tile([NB, D], F32, tag="s2", name="s2", bufs=2)
            nc.vector.tensor_add(s2[sl, :], s1[sl, :], rr[sl, 2 * D:3 * D])
            ot = epi.tile([NB, D], F32, tag="ot", name="ot", bufs=2)
            nc.vector.tensor_scalar_mul(ot[sl, :], s2[sl, :],
                                        float(gamma[i]) / 3.0)
            fo = epi.tile([NB, D], F32, tag="fo", name="fo", bufs=2)
            nc.vector.tensor_add(fo[sl, :], ot[sl, :], fn[sl, :])
            nc.sync.dma_start(feats_d[i, sl, :], fo[sl, :])

        # sections by gg: first consume partition groups {0,1}, then {2,3};
        # stage groups of 4 pairs = (g in section) x (r, r+1).
        # Outer products are emitted one pair ahead of their exp so the PE
        # stream runs [outer(n+1), ZW(n)] inside exp(n)'s shadow.
        seq = []
        for gg in range(2):
            for u in range(NR // 2):
                for k in range(4):
                    g2, dr = k % 2, k // 2
                    g, r = 2 * gg + g2, 2 * u + dr
                    seq.append((g, r))

        def emit_outer(idx):
            g, r = seq[idx]
            et = psA.tile([128, 1536], F32, tag="big", name="et")
            for (h, jp0, w, ec) in OUTER_CHUNKS:
                nc.tensor.matmul(
                    et[:, ec:ec + w],
                    kpack[32 * g:32 * g + 1,
                          D * r + 128 * h:D * r + 128 * h + 128],
                    qpack[32 * g:32 * g + 1,
                          JP * r + jp0:JP * r + jp0 + w],
                    start=True, stop=True,
                    tile_position=(32 * g, 0))
            return et

        pend = {0: emit_outer(0)}
        exs = {}
        stages = {}

        def emit_zw(jdx):
            # ZW + copies for pair jdx (one iteration behind its exp, so the
            # PE stream meets each exp's outer products first)
            g, r = seq[jdx]
            n = 4 * r + g
            k = jdx % 4
            if k == 0:
                stages[jdx // 4] = stg.tile([2, 4 * JP], F32, tag="stage",
                                            name="stage")
            stage = stages[jdx // 4]
            ex = exs.pop(jdx)
            lhs = [vones[i][h][:, 2 * n:2 * n + 2] for h in range(2)]
            zwA = psB.tile([2, 512], F32, tag="zwA", name="zwA")
            nc.tensor.matmul(zwA[:], lhs[0], ex[:, 0:512],
                             start=True, stop=False, skip_group_check=True)
            nc.tensor.matmul(zwA[:], lhs[1], ex[:, 768:1280],
                             start=False, stop=True, skip_group_check=True)
            zwB = psB.tile([2, 256], F32, tag="zwB", name="zwB")
            nc.tensor.matmul(zwB[:], lhs[0], ex[:, 512:768],
                             start=True, stop=False, skip_group_check=True)
            nc.tensor.matmul(zwB[:], lhs[1], ex[:, 1280:1536],
                             start=False, stop=True, skip_group_check=True)
            nc.vector.tensor_copy(stage[:, JP * k:JP * k + 512], zwA[:])
            nc.vector.tensor_copy(stage[:, JP * k + 512:JP * k + 768], zwB[:])
            if k == 3:
                gg, u = jdx // 32, (jdx % 32) // 4
                base = 8 * u + 2 * gg
                for c in range(2):
                    nc.sync.dma_start(
                        zwbuf[base + 4 * c:base + 4 * c + 2, 0:JP],
                        stage[0:1, 2 * c * JP:(2 * c + 2) * JP]
                        .rearrange("p (k f) -> p k f", k=2))
                    nc.sync.dma_start(
                        zwbuf[base + 4 * c:base + 4 * c + 2, JP:2 * JP],
                        stage[1:2, 2 * c * JP:(2 * c + 2) * JP]
                        .rearrange("p (k f) -> p k f", k=2))
                del stages[jdx // 4]
                if gg == 1 and u % 4 == 3:
                    epilogue_chunk(u // 4)
                if inject is not None and jdx == 39:
                    inject()

        for idx in range(len(seq)):
            et = pend.pop(idx)
            ex = expp.tile([128, 1536], F32R, tag="expE", name="ex", bufs=3)
            nc.scalar.activation(ex[:], et[:], AF.Exp)
            exs[idx] = ex
            if idx + 1 < len(seq):
                pend[idx + 1] = emit_outer(idx + 1)
            if idx >= 1:
                emit_zw(idx - 1)
        emit_zw(len(seq) - 1)

    prologue(0)
    for i in range(NJ):
        nxt = (lambda j: (lambda: prologue(j)))(i + 1) if i + 1 < NJ else None
        main_phase(i, inject=nxt)


def build_nc(gamma, repeat=1):
    nc = bacc.Bacc("TRN2", target_bir_lowering=False, debug=False,
                   num_devices=N_CORES)
    ftp_d = nc.dram_tensor("ftp", [128, 3 * NJ, NB], F32R, kind="ExternalInput").ap()
    wq1_d = nc.dram_tensor("wq1", [128, 3 * NJ, D], F32R, kind="ExternalInput").ap()
    wk1_d = nc.dram_tensor("wk1", [128, 3 * NJ, D], F32R, kind="ExternalInput").ap()
    wv1_d = nc.dram_tensor("wv1", [128, 3 * NJ, D], F32R, kind="ExternalInput").ap()
    fnat_d = nc.dram_tensor("fnat", [NJ, NB, D], F32, kind="ExternalInput").ap()
    vinit_d = nc.dram_tensor("vinit", [128, 2 * NB], F32R, kind="ExternalInput").ap()
    feats_d = nc.dram_tensor("feats", [NJ, NB, D], F32, kind="ExternalOutput").ap()
    meansp_d = nc.dram_tensor("meansp", [NJ, 128], F32, kind="ExternalOutput").ap()
    aps = (ftp_d, wq1_d, wk1_d, wv1_d, fnat_d, vinit_d, feats_d, meansp_d)

    with tile.TileContext(nc) as tc:
        with tc.tile_pool(name="io", bufs=1) as io, \
             tc.tile_pool(name="pro", bufs=1) as pro, \
             tc.tile_pool(name="mainq", bufs=1) as mainq, \
             tc.tile_pool(name="expp", bufs=2) as expp, \
             tc.tile_pool(name="stg", bufs=2) as stg, \
             tc.tile_pool(name="epi", bufs=1) as epi, \
             tc.tile_pool(name="psA", bufs=2, space="PSUM") as psA, \
             tc.tile_pool(name="psB", bufs=1, space="PSUM") as psB, \
             tc.tile_pool(name="dram", bufs=1, space="DRAM") as dram:
            pools = (io, pro, mainq, expp, stg, epi, psA, psB, dram)
            for _ in range(repeat):
                _emit(nc, tc, pools, aps, gamma)
    nc.compile()
    return nc


def pack_inputs(feats_list, Wq, bq, Wk, bk, Wv, bv, b0):
    ftp = np.zeros((128, 3 * NJ, NB), np.float32)
    for j in range(NJ):
        FjT = feats_list[j][b0:b0 + NB].T
        ftp[:, 3 * j + 0, :] = FjT[0:128]
        ftp[:, 3 * j + 1, :] = FjT[128:256]
        ftp[0, 3 * j + 2, :] = 1.0

    def w1(W, b):
        out = np.zeros((128, 3 * NJ, D), np.float32)
        for i in range(NJ):
            WT = W[i].T
            out[:, 3 * i + 0, :] = WT[0:128]
            out[:, 3 * i + 1, :] = WT[128:256]
            out[0, 3 * i + 2, :] = b[i]
        return out

    fnat = np.stack([f[b0:b0 + NB] for f in feats_list]).astype(np.float32)
    return {
        "ftp": ftp.reshape(128, -1),
        "wq1": w1(Wq, bq).reshape(128, -1),
        "wk1": w1(Wk, bk).reshape(128, -1),
        "wv1": w1(Wv, bv).reshape(128, -1),
        "fnat": fnat,
        "vinit": np.ones((128, 2 * NB), np.float32),
    }


def postprocess(results):
    feats = np.concatenate([np.asarray(r["feats"]) for r in results], axis=1)
    tot = np.zeros(NJ, np.float64)
    for r in results:
        tot += np.asarray(r["meansp"])[:, 0:NB].astype(np.float64).sum(axis=1)
    means = tot / (NJ * B * D * D)
    e = np.exp(means - means.max())
    alphas = (e / e.sum()).astype(np.float32)
    return np.ascontiguousarray(feats.astype(np.float32)), alphas


_nc_cache = {}


def get_nc(gamma, repeat=1):
    key = (bytes(np.asarray(gamma, np.float32).tobytes()), repeat)
    if key not in _nc_cache:
        _nc_cache[key] = build_nc(gamma, repeat=repeat)
    return _nc_cache[key]


def kernel(feat0, feat1, feat2, Wq, bq, Wk, bk, Wv, bv, gamma):
    feats_list = [np.asarray(feat0, np.float32), np.asarray(feat1, np.float32),
                  np.asarray(feat2, np.float32)]
    Wq, bq = np.asarray(Wq, np.float32), np.asarray(bq, np.float32)
    Wk, bk = np.asarray(Wk, np.float32), np.asarray(bk, np.float32)
    Wv, bv = np.asarray(Wv, np.float32), np.asarray(bv, np.float32)
    gamma = np.asarray(gamma, np.float32)

    nc = get_nc(gamma)
    in_maps = [pack_inputs(feats_list, Wq, bq, Wk, bk, Wv, bv, NB * c)
               for c in range(N_CORES)]
    res = run_bass_kernel_spmd(nc, in_maps, core_ids=list(range(N_CORES)))
    return postprocess(res.results)
